# revision 42
# baseline (speedup 1.0000x reference)
"""Single-head causal attention on 8 Trainium2 NeuronCores.

Problem: x[8, 2048, 1024] f32, per-head projections (H=64), causal
softmax attention scaled by C**-0.5.

Strategy: data-parallel over batch (1 batch element per core). Per core
everything is kept in transposed layout so no fp32 on-chip transposes of
large tensors are needed:

  - projection inputs (x, wq|wk, wv) travel as fp8-e4m3: the start of
    the kernel is HBM-bound, and fp8 halves the x bytes (fp8 matmul runs
    at bf16 speed on the PE, and the projection outputs accumulate in
    f32 PSUM). The C**-0.5 logit scale is folded into the exp's free
    affine (exp(s/32)) because pre-scaled fp8 weights would underflow.
    V for the first 512 tokens is additionally computed from an f16 copy
    (early rows average few V values, so fp8 V error would not wash out)
  - host prepacks x wave-major [128, 4 waves, 8 c-chunks, 512 tok] so
    each wave loads as one DMA with 4-8KB contiguous partition lines
  - dummy matmuls sized to end when the first x data lands keep the PE
    HAM clock-gate warm (K=8/8, 2.4 GHz) for the real projections
  - Q^T/K^T [64, T] computed with [wq|wk] as stationary into one PSUM
    tile via PE column tiling; epilogue writes qk=[Q;K] in one 128-lane
    DVE op (bias fused), plus two small copies to build kq=[K;Q] for
    the row-tiled S matmuls
  - V^T for two token chunks is computed in one column-tiled pass
    (different moving streams -> both column groups run concurrently)
  - kernel pipelined over T-chunks of 512: projections for chunk tj
    weave with attention for query chunk tj-1
  - S^T tiles [128 Tk, 512 Tq] = (K^T chunk).T @ Q^T, two at a time via
    PE row tiling into one two-bank PSUM tile, single ScalarE exp per
    pair; causal mask via 0/1 mask multiply on VectorE
  - P@V: V tiles [128 Tk, 65] carry an appended ones column, so the
    softmax denominator falls out of the same PSUM accumulation
  - y returned as f16 [65, T]; normalization + transpose on host

Outputs are returned as float32 [8, 2048, 64].
"""

import numpy as np
import ml_dtypes

import concourse.bass as bass
import concourse.mybir as mybir
import concourse.tile as tile
from concourse import bacc
from concourse.bass_utils import run_bass_kernel_spmd

B, T, C, H = 8, 2048, 1024, 64
N_CORES = 8
TQ = 512          # Tq chunk (one fp32 PSUM bank)
N_JQ = T // TQ    # 4
N_TK = T // 128   # 16
N_KC = C // 128   # 8  contraction chunks for projections

DT16 = mybir.dt.float16
DT8 = mybir.dt.float8e4
F32 = mybir.dt.float32
AF = mybir.ActivationFunctionType

SCALE = 1.0 / 32.0   # C**-0.5, applied inside the exp
HP = 80              # DoubleRow stationary free dim (k-tile step must be 16B-aligned)
W_WARM = 12          # HAM warmup dummy matmuls (512 cols each)

_CACHED_NC = None


def build_program(reps=1):
    nc = bacc.Bacc("TRN2", target_bir_lowering=False, debug=False,
                   num_devices=N_CORES)

    xw8_d = nc.dram_tensor("xw8", [128, N_JQ, N_KC, TQ], DT8,
                           kind="ExternalInput").ap()
    x16_d = nc.dram_tensor("x16", [128, N_KC, TQ], DT16,
                           kind="ExternalInput").ap()
    wqk_d = nc.dram_tensor("wqk", [128, N_KC, 128], DT8,
                           kind="ExternalInput").ap()
    wv8_d = nc.dram_tensor("wv8", [128, N_KC, H], DT8,
                           kind="ExternalInput").ap()
    wv16_d = nc.dram_tensor("wv16", [128, N_KC, H], DT16,
                            kind="ExternalInput").ap()
    bqk_d = nc.dram_tensor("bqk", [128, 1], F32, kind="ExternalInput").ap()
    bv_d = nc.dram_tensor("bv", [128, 1], F32, kind="ExternalInput").ap()
    # additive causal masks (0 allowed / -240 masked) per diagonal
    # offset m*128; applied inside the PE via (240*I) @ mask = -57600
    masks_d = nc.dram_tensor("masks", [128, 4, TQ], DT8,
                             kind="ExternalInput").ap()
    id240_d = nc.dram_tensor("id240", [128, 128], DT8,
                             kind="ExternalInput").ap()
    ident_d = nc.dram_tensor("ident", [128, H], DT16, kind="ExternalInput").ap()
    y_d = nc.dram_tensor("y", [H + 1, T], DT16, kind="ExternalOutput").ap()

    with tile.TileContext(nc) as tc:
        with (
            tc.tile_pool(name="const", bufs=1) as const,
            tc.tile_pool(name="data", bufs=1) as data,
            tc.tile_pool(name="et", bufs=6) as et_pool,
            tc.tile_pool(name="et16", bufs=1) as et16_pool,
            tc.tile_pool(name="ysb", bufs=2) as y_pool,
            tc.tile_pool(name="ps_proj", bufs=2, space="PSUM") as ps_proj,
            tc.tile_pool(name="ps_s", bufs=2, space="PSUM") as ps_s,
            tc.tile_pool(name="ps_o", bufs=2, space="PSUM") as ps_o,
        ):
            # ---- constants ----------------------------------------------
            wqk_sb = const.tile([128, N_KC, 128], DT8, tag="wqk")
            wv8_sb = const.tile([128, N_KC, H], DT8, tag="wv8")
            wv16_sb = const.tile([128, N_KC, H], DT16, tag="wv16")
            bqk_sb = const.tile([128, 1], F32, tag="bqk")
            bv_sb = const.tile([128, 1], F32, tag="bv")
            masks_sb = const.tile([128, 4, TQ], DT8, tag="masks")
            id240_sb = const.tile([128, 128], DT8, tag="id240")
            ident_sb = const.tile([128, H], DT16, tag="ident")
            warm_sb = const.tile([128, TQ], DT16, tag="warm")

            # const loads on the gpsimd queue (otherwise idle), smallest-
            # deadline first; masks01 last (needed ~20us in)
            nc.gpsimd.dma_start(wqk_sb[:], wqk_d)
            nc.gpsimd.dma_start(bqk_sb[:], bqk_d)
            nc.gpsimd.dma_start(bv_sb[:], bv_d)
            nc.gpsimd.dma_start(wv8_sb[:], wv8_d)
            nc.gpsimd.dma_start(wv16_sb[:], wv16_d)
            nc.gpsimd.dma_start(ident_sb[:], ident_d)

            # ---- per-iteration tiles ------------------------------------
            xT_sb = data.tile([128, N_JQ, N_KC, TQ], DT8, tag="xT")
            x16_sb = data.tile([128, N_KC, TQ], DT16, tag="x16")
            qk_sb = data.tile([128, T], DT16, tag="qk")   # 0:64 Q, 64:128 K
            kq_sb = data.tile([128, T], DT16, tag="kq")   # 0:64 K, 64:128 Q
            vT_sb = data.tile([128, 2, TQ], DT16, tag="vT")
            # f16 V for k-tiles 0,1 (attn(0)'s first PV pair stays f16 for
            # early-row accuracy); fp8 k-tile pairs [pair, j, H+1] for the
            # DoubleRow PV matmuls (ones column at H)
            v_sb = data.tile([128, 2, H + 1], DT16, tag="v")
            v2_sb = data.tile([128, N_TK // 2, 2, HP], DT8, tag="v2")
            nc.vector.memset(warm_sb[:], 0.5)
            nc.vector.memset(v_sb[:], 1.0)
            nc.vector.memset(v2_sb[:], 1.0)

            # ---- x DMAs -------------------------------------------------
            def dma_x(wave):
                def go():
                    if wave == 0:
                        nc.sync.dma_start(xT_sb[:, 0, 0:4], xw8_d[:, 0, 0:4])
                        nc.scalar.dma_start(xT_sb[:, 0, 4:8], xw8_d[:, 0, 4:8])
                    elif wave == 2:
                        nc.scalar.dma_start(xT_sb[:, wave], xw8_d[:, wave])
                    else:
                        nc.sync.dma_start(xT_sb[:, wave], xw8_d[:, wave])
                return go

            def dma_x16():
                nc.sync.dma_start(x16_sb[:, 0:4], x16_d[:, 0:4])
                nc.scalar.dma_start(x16_sb[:, 4:8], x16_d[:, 4:8])

            def dma_masks():
                nc.sync.dma_start(masks_sb[:], masks_d)
                nc.scalar.dma_start(id240_sb[:], id240_d)

            # ---- HAM warmup: keep PE busy until first x data lands ------
            def warmup():
                wps = ps_proj.tile([128, TQ], F32, tag="proj", name="warm_ps")
                for _w in range(W_WARM):
                    nc.tensor.matmul(wps[:], warm_sb[:, 0:128], warm_sb[:],
                                     start=True, stop=True,
                                     skip_group_check=True)

            # ---- projections -------------------------------------------
            def qk_thunks(tj):
                sl = slice(tj * TQ, (tj + 1) * TQ)
                st = {}
                th = []

                def qk_mm(c):
                    if c == 0:
                        st["ps"] = ps_proj.tile([128, TQ], F32, tag="proj",
                                                name="ps_qk")
                    ps = st["ps"]
                    nc.tensor.matmul(ps[0:64, :], wqk_sb[:, c, 0:64],
                                     xT_sb[:, tj, c, :],
                                     start=(c == 0), stop=(c == N_KC - 1),
                                     tile_position=(0, 0),
                                     skip_group_check=True)
                    nc.tensor.matmul(ps[64:128, :], wqk_sb[:, c, 64:128],
                                     xT_sb[:, tj, c, :],
                                     start=(c == 0), stop=(c == N_KC - 1),
                                     tile_position=(0, 64),
                                     skip_group_check=True)
                for c in range(N_KC):
                    th.append(lambda c=c: qk_mm(c))

                def qk_epi():
                    nc.vector.tensor_scalar_add(qk_sb[:, sl], st["ps"][:],
                                                bqk_sb[:])
                    nc.vector.tensor_copy(kq_sb[0:64, sl], qk_sb[64:128, sl])
                    nc.vector.tensor_copy(kq_sb[64:128, sl], qk_sb[0:64, sl])
                th.append(qk_epi)
                return th

            def vpair_thunks(p):
                # V for token chunks 2p (rows 0:64) and 2p+1 (rows 64:128);
                # chunk 0 streams the f16 copy, the rest fp8
                st = {}
                th = []

                def v_mm(c):
                    if c == 0:
                        st["psv"] = ps_proj.tile([128, TQ], F32, tag="proj",
                                                 name="ps_v")
                    if p == 0:
                        nc.tensor.matmul(st["psv"][0:64, :], wv16_sb[:, c, :],
                                         x16_sb[:, c, :],
                                         start=(c == 0), stop=(c == N_KC - 1),
                                         tile_position=(0, 0),
                                         skip_group_check=True)
                    else:
                        nc.tensor.matmul(st["psv"][0:64, :], wv8_sb[:, c, :],
                                         xT_sb[:, 2 * p, c, :],
                                         start=(c == 0), stop=(c == N_KC - 1),
                                         tile_position=(0, 0),
                                         skip_group_check=True)
                    nc.tensor.matmul(st["psv"][64:128, :], wv8_sb[:, c, :],
                                     xT_sb[:, 2 * p + 1, c, :],
                                     start=(c == 0), stop=(c == N_KC - 1),
                                     tile_position=(0, 64),
                                     skip_group_check=True)
                for c in range(N_KC):
                    th.append(lambda c=c: v_mm(c))

                def v_epi():
                    nc.vector.tensor_scalar_add(vT_sb[:, p, :], st["psv"][:],
                                                bv_sb[:])
                th.append(v_epi)

                def v_tr(r):
                    i = p * 8 + r
                    half = r // 4       # 0: chunk 2p, 1: chunk 2p+1
                    src = vT_sb[64 * half:64 * half + 64, p,
                                (r % 4) * 128:(r % 4 + 1) * 128]
                    pt = ps_proj.tile([128, H], DT16, tag="proj")
                    nc.tensor.transpose(
                        pt, src, ident_sb[64 * half:64 * half + 64, :])
                    nc.vector.tensor_copy(v2_sb[:, i // 2, i % 2, 0:H], pt)
                    if i < 2:
                        nc.vector.tensor_copy(v_sb[:, i, 0:H], pt)
                for r in range(8):
                    th.append(lambda r=r: v_tr(r))
                return th

            # ---- attention ---------------------------------------------
            def attn_thunks(jq):
                sl = slice(jq * TQ, (jq + 1) * TQ)
                n_ik = (jq + 1) * 4
                st = {}
                th = []

                def s_pair(s):
                    if s == 0:
                        st["po"] = ps_o.tile([HP, TQ], F32, tag="out",
                                             name="po")
                    ps2 = ps_s.tile([128, 2, TQ], F32, tag="s")
                    st["ps2"] = ps2
                    for half, ik in ((0, 2 * s), (1, 2 * s + 1)):
                        m = ik - jq * 4
                        masked = m >= 0
                        if half == 0:
                            # stationary K on partitions 0:64, moving Q 0:64
                            nc.tensor.matmul(
                                ps2[:, half, :],
                                kq_sb[0:64, ik * 128:(ik + 1) * 128],
                                qk_sb[0:64, sl],
                                start=True, stop=not masked,
                                tile_position=(0, 0))
                        else:
                            # stationary K on partitions 64:128, moving Q 64:128
                            nc.tensor.matmul(
                                ps2[:, half, :],
                                qk_sb[64:128, ik * 128:(ik + 1) * 128],
                                kq_sb[64:128, sl],
                                start=True, stop=not masked,
                                tile_position=(64, 0))
                        if masked:
                            nc.tensor.matmul(
                                ps2[:, half, :], id240_sb[:],
                                masks_sb[:, m, :],
                                start=False, stop=True,
                                tile_position=(0, 0))

                def exp_pair(s):
                    f16_pair = jq == 0 and s == 0
                    if f16_pair:
                        eT = et16_pool.tile([128, 2, TQ], DT16, tag="et16",
                                            name="eT16")
                    else:
                        eT = et_pool.tile([128, 2, TQ], DT8, tag="et",
                                          name="eT")
                    st[f"eT{s}"] = eT
                    nc.scalar.activation(eT[:], st["ps2"][:], AF.Exp,
                                         scale=SCALE)

                def pv_pair(s):
                    ik_a, ik_b = 2 * s, 2 * s + 1
                    eT = st[f"eT{s}"]
                    if jq == 0 and s == 0:
                        nc.tensor.matmul(st["po"][0:H + 1, :], v_sb[:, 0, :],
                                         eT[:, 0, :],
                                         start=True, stop=False)
                        nc.tensor.matmul(st["po"][0:H + 1, :], v_sb[:, 1, :],
                                         eT[:, 1, :],
                                         start=False, stop=(n_ik == 2))
                    else:
                        nc.tensor.matmul(st["po"], v2_sb[:, s], eT[:],
                                         start=(ik_a == 0),
                                         stop=(ik_b == n_ik - 1),
                                         perf_mode=mybir.MatmulPerfMode
                                         .DoubleRow)

                for s in range(n_ik // 2):
                    th.append(lambda s=s: (s_pair(s), exp_pair(s)))
                    th.append(lambda s=s: pv_pair(s))

                def yout():
                    y_sb = y_pool.tile([H + 1, TQ], DT16, tag="ysb")
                    nc.vector.tensor_copy(y_sb[:], st["po"][0:H + 1, :])
                    nc.gpsimd.dma_start(y_d[:, sl], y_sb[:])
                th.append(yout)
                return th

            def weave(a, b, f=1.0):
                out, i, j = [], 0, 0
                na, nb = len(a), len(b)
                while i < na or j < nb:
                    if j >= nb or (i < na and i * nb <= j * na * f):
                        out.append(a[i]); i += 1
                    else:
                        out.append(b[j]); j += 1
                return out

            # ---- schedule ----------------------------------------------
            # NOTE: emission order defines data-dependency semantics;
            # attention for chunk jq must be emitted strictly after the
            # projections (and V transposes) it consumes.
            dma_x(0)()
            dma_x(1)()
            dma_x16()
            warmup()
            a0 = attn_thunks(0)   # [s0, pv0, s1, pv1, yout]
            vp0 = vpair_thunks(0)  # [v_mm x8, vepi, v_tr x8]
            vp1 = vpair_thunks(1)
            a1 = attn_thunks(1)
            a2 = attn_thunks(2)
            a3 = attn_thunks(3)
            qk2 = qk_thunks(2)
            qk3 = qk_thunks(3)
            # Front-load projections: qk(1) directly after qk(0) keeps
            # the PE busy while the DVE runs epi(0); attn(0)'s S/exp
            # start the ScalarE chain as early as possible. After that,
            # one flat emission list paces the remaining 18 S-pairs at
            # the ACT cadence (~1.1us of PE work apart) with PV / QK /
            # V-transposes as filler between them.
            pre = (qk_thunks(0) + qk_thunks(1) + [dma_masks, dma_x(2)]
                   + [a0[0], a0[2]] + vp0)
            flat = (
                [dma_x(3)]
                + qk2
                + [a1[0], a0[1], a1[2]]
                + qk3[0:4]
                + [a0[3], a0[4], a1[4]]
                + qk3[4:9]
                + [a1[1], a1[6]]
                + vp1[0:4]
                + [a1[3], a2[0]]
                + vp1[4:9]
                + [a1[5], a2[2]]
                + vp1[9:13]
                + [a1[7], a1[8], a2[4]]
                + vp1[13:17]
                + [a2[1], a2[6]]
                + [a2[3], a2[8], a2[5], a3[0], a2[10], a2[7], a3[2],
                   a2[9], a2[11], a2[12],
                   a3[4], a3[1], a3[6], a3[3], a3[8], a3[5], a3[10],
                   a3[7], a3[12], a3[9], a3[14], a3[11], a3[13],
                   a3[15], a3[16]]
            )
            for t in pre + flat:
                t()

    nc.compile()
    return nc


def prepare_in_maps(x, wq, bq, wk, bk, wv, bv):
    f16 = np.float16
    f32 = np.float32
    f8 = ml_dtypes.float8_e4m3
    x = np.asarray(x)
    # wave-major prepack: xw[p, w, c, t] = x[b, w*512+t, c*128+p]
    xw8 = x.astype(f8).reshape(B, N_JQ, TQ, N_KC, 128).transpose(0, 4, 1, 3, 2)
    # f16 copy of the first 512 tokens (for V accuracy in early rows)
    x16 = x[:, :TQ].astype(f16).reshape(B, TQ, N_KC, 128).transpose(0, 3, 2, 1)
    wqk = np.concatenate([np.asarray(wq), np.asarray(wk)], 1).astype(f8)
    wqk = np.ascontiguousarray(wqk.reshape(N_KC, 128, 128).transpose(1, 0, 2))
    wv_m = np.asarray(wv)
    wv8 = np.ascontiguousarray(
        wv_m.astype(f8).reshape(N_KC, 128, H).transpose(1, 0, 2))
    wv16 = np.ascontiguousarray(
        wv_m.astype(f16).reshape(N_KC, 128, H).transpose(1, 0, 2))
    bqk = np.concatenate([np.asarray(bq), np.asarray(bk)]) \
        .astype(f32).reshape(128, 1)
    bv_c = np.concatenate([np.asarray(bv), np.asarray(bv)]) \
        .astype(f32).reshape(128, 1)
    tk_i = np.arange(128)[:, None]
    tq_i = np.arange(TQ)[None, :]
    masks = np.ascontiguousarray(np.stack(
        [np.where(tq_i >= tk_i + m * 128, 0.0, -240.0) for m in range(4)],
        1).astype(f8))
    id240 = (np.eye(128) * 240.0).astype(f8)
    ident = np.ascontiguousarray(
        np.concatenate([np.eye(H), np.eye(H)], 0).astype(f16))
    shared = {"wqk": wqk, "wv8": wv8, "wv16": wv16, "bqk": bqk, "bv": bv_c,
              "masks": masks, "id240": id240, "ident": ident}
    return [{"xw8": np.ascontiguousarray(xw8[b]),
             "x16": np.ascontiguousarray(x16[b]), **shared}
            for b in range(B)]


def postprocess(ys):
    out = np.empty((B, T, H), np.float32)
    for b, y in enumerate(ys):
        yf = y.astype(np.float32)
        out[b] = (yf[:H] / yf[H:H + 1]).T
    return out


def kernel(**inputs):
    global _CACHED_NC
    if _CACHED_NC is None:
        _CACHED_NC = build_program(reps=1)
    nc = _CACHED_NC
    in_maps = prepare_in_maps(
        inputs["x"], inputs["wq"], inputs["bq"], inputs["wk"],
        inputs["bk"], inputs["wv"], inputs["bv"])
    res = run_bass_kernel_spmd(nc, in_maps, core_ids=list(range(N_CORES)))
    return postprocess([r["y"] for r in res.results])


if __name__ == "__main__":
    rng = np.random.default_rng(0)
    demo = {
        "x": rng.standard_normal((B, T, C), dtype=np.float32),
        "wq": rng.standard_normal((C, H), dtype=np.float32) * 0.02,
        "bq": rng.standard_normal((H,), dtype=np.float32) * 0.02,
        "wk": rng.standard_normal((C, H), dtype=np.float32) * 0.02,
        "bk": rng.standard_normal((H,), dtype=np.float32) * 0.02,
        "wv": rng.standard_normal((C, H), dtype=np.float32) * 0.02,
        "bv": rng.standard_normal((H,), dtype=np.float32) * 0.02,
    }
    out = kernel(**demo)
    print("kernel output:", out.shape, out.dtype)
